# revision 13
# baseline (speedup 1.0000x reference)
"""Trainium2 Bass kernel for ChebyshevActivation.

Math:
    scale = clip(input_scale, 0.1, 2.0)
    t = tanh(x * scale)                        # t in (-1, 1)
    out[b, o] = sum_w coeffs[o, w] * sum_i T_w(t[b, i])

Since |t| < 1, all Chebyshev T_n(t) lie in [-1, 1] and the reference's
clip(+-100) is dead code.  We work in the monomial basis: with power sums
M_j[b] = sum_i t[b,i]^j (M_0 = IN_F exactly) and G = coeffs @ C (C the
Chebyshev->monomial matrix), out = M @ G^T.

Power-sum extraction is one fused pass per "piece": ACT squares with
accum_out, or DVE scalar_tensor_tensor with accum_out (this environment's
walrus rejects TensorScalarPtr on GPSIMD and raw-ISA custom-DVE encodings
from plain Bass, so pieces live on ACT/DVE and the module is built with
Bacc).  Every moment can be split column-wise into pieces on different
engines; each piece accumulates into its own column of the per-tile moment
matrix, and the host duplicates the matching G rows so the final PE matmul
(K = #pieces+1) re-merges them.  Channels t1..t4 are fp16 (bf16 loses too
much precision through the basis change; fp16 keeps DVE 2x modes).

Per-core layout: data-parallel over batch, 8 cores x 1024 rows,
8 row-tiles of [128, 2048] per core.
"""

import numpy as np

import concourse.bass as bass
import concourse.bacc as bacc
import concourse.mybir as mybir
import concourse.tile as tile
from concourse import masks
from concourse.bass_utils import run_bass_kernel_spmd

# This environment's walrus build rejects raw client-encoded ISA instructions
# ("ISA wrong length" for the 64-byte EVENT_SEMAPHORE_RANGE_CLEAR emitted by
# the TileContext exit barrier).  The clear only matters when re-executing an
# already-loaded NEFF; each kernel() call compiles+loads+runs once, so skip it.
bass.BassGpSimd.sem_clear = lambda self, r: None

N_CORES = 8
BATCH = 8192
IN_F = 2048
OUT_F = 1024
DEG = 8
W = DEG + 1  # 9 moments
ROWS_PER_CORE = BATCH // N_CORES  # 1024
P = 128
NTILES = ROWS_PER_CORE // P  # 8

F32 = mybir.dt.float32
F16 = mybir.dt.float16
MULT = mybir.AluOpType.mult
ADD = mybir.AluOpType.add
SQUARE = mybir.ActivationFunctionType.Square
TANH = mybir.ActivationFunctionType.Tanh

# Stream definitions: name -> (in0, in1, value_dst) ; value_dst None => junk.
# in0 == in1 means the stream is a square (ACT-eligible).
STREAMS = {
    "t2": ("t1", "t1", "t2"),
    "t3": ("t2", "t1", "t3"),
    "t4": ("t2", "t2", "t4"),
    "M5": ("t4", "t1", None),
    "M6": ("t3", "t3", None),
    "M7": ("t4", "t3", None),
    "M8": ("t4", "t4", None),
}
STREAM_MOMENT = {"t2": 2, "t3": 3, "t4": 4, "M5": 5, "M6": 6, "M7": 7, "M8": 8}

# Engine assignment config: stream -> list of (engine, fraction).
# Engines: "A" = ACT square (squares only), "D" = DVE TTR, "G" = GPSIMD stt.
CFG = {
    "t2": [("D", 1.0)],
    "t3": [("D", 1.0)],
    "t4": [("A", 1.0)],
    "M5": [("D", 1.0)],
    "M6": [("A", 1.0)],
    "M7": [("D", 1.0)],
    "M8": [("A", 1.0)],
    "oc_act": 1.0,   # fraction of the PSUM->SBUF output copy done on ACT
    "mt": "D",       # moment-transpose PSUM->SBUF copy engine
    "xin_bufs": 4,
    "chan_bufs": 3,
    "ostage_bufs": 3,
}


def _cheb_monomial_matrix(deg=DEG):
    C = np.zeros((deg + 1, deg + 1), dtype=np.float64)
    C[0, 0] = 1.0
    if deg >= 1:
        C[1, 1] = 1.0
    for n in range(2, deg + 1):
        C[n, 1:] = 2.0 * C[n - 1, :-1]
        C[n, :] -= C[n - 2, :]
    return C


def _pieces(cfg):
    """Deterministic piece list: (stream, engine, col_lo, col_hi)."""
    out = []
    enabled = cfg.get("only_streams")
    for s in STREAMS:
        if enabled is not None and s not in enabled:
            continue
        cols = 0
        parts = cfg[s]
        for idx, (eng, frac) in enumerate(parts):
            if idx == len(parts) - 1:
                hi = IN_F
            else:
                hi = cols + int(round(IN_F * frac / 128.0)) * 128
                hi = min(hi, IN_F)
            if hi > cols:
                out.append((s, eng, cols, hi))
            cols = hi
    return out


def _moment_rows(cfg):
    """Row j of GT corresponds to these moments: [0 (M0), 1 (M1 tanh), *pieces]."""
    rows = [0, 1]
    for s, _eng, _lo, _hi in _pieces(cfg):
        rows.append(STREAM_MOMENT[s])
    return rows


def _build_nc(scale: float, cfg=CFG) -> bass.Bass:
    pieces = _pieces(cfg)
    K = 2 + len(pieces)  # M0 + M1 + pieces
    assert K <= 24
    mcols = K

    nc = bacc.Bacc("TRN2")
    x = nc.dram_tensor("x", [ROWS_PER_CORE, IN_F], F32, kind="ExternalInput")
    gt = nc.dram_tensor("gt", [K, OUT_F], F32, kind="ExternalInput")
    out = nc.dram_tensor("out", [ROWS_PER_CORE, OUT_F], F32, kind="ExternalOutput")

    with tile.TileContext(nc) as tc:
        with (
            tc.tile_pool(name="singles", bufs=1) as singles,
            tc.tile_pool(name="xin", bufs=cfg["xin_bufs"]) as xin,
            tc.tile_pool(name="chan", bufs=cfg["chan_bufs"]) as chan,
            tc.tile_pool(name="junk", bufs=1) as junkp,
            tc.tile_pool(name="mpool", bufs=4) as mpool,
            tc.tile_pool(name="mtsb", bufs=4) as mtsb,
            tc.tile_pool(name="ostage", bufs=cfg["ostage_bufs"]) as ostage,
            tc.tile_pool(name="pt", bufs=2, space="PSUM") as pt,
            tc.tile_pool(name="pout", bufs=2, space="PSUM") as pout,
        ):
            gt_sb = singles.tile([K, OUT_F], F32)
            nc.sync.dma_start(out=gt_sb[:, :], in_=gt[:, :])
            ident = singles.tile([P, P], F32)
            masks.make_identity(nc, ident[:, :])

            j_dve = junkp.tile([P, IN_F], F16, tag="jd")
            j_act = junkp.tile([P, IN_F], F16, tag="ja")
            j_gps = junkp.tile([P, IN_F], F16, tag="jg")
            JUNK = {"A": j_act, "D": j_dve, "G": j_gps}

            for it in range(NTILES):
                x_t = xin.tile([P, IN_F], F32)
                nc.sync.dma_start(out=x_t[:, :], in_=x[it * P:(it + 1) * P, :])

                m_t = mpool.tile([P, mcols], F32)
                nc.gpsimd.memset(m_t[:, 0:1], float(IN_F))

                t1 = chan.tile([P, IN_F], F16, tag="t1")
                t2 = chan.tile([P, IN_F], F16, tag="t2")
                t3 = chan.tile([P, IN_F], F16, tag="t3")
                t4 = chan.tile([P, IN_F], F16, tag="t4")
                VALS = {"t1": t1, "t2": t2, "t3": t3, "t4": t4}

                # t1 = tanh(scale * x), accum -> M1 (col 1)
                nc.scalar.activation(
                    out=t1[:, :], in_=x_t[:, :], func=TANH,
                    scale=scale, accum_out=m_t[:, 1:2],
                )

                for pidx, (s, eng, lo, hi) in enumerate(pieces):
                    a_name, b_name, dst_name = STREAMS[s]
                    a = VALS[a_name]
                    b = VALS[b_name]
                    dst = VALS[dst_name] if dst_name else JUNK[eng]
                    mcol = m_t[:, 2 + pidx:3 + pidx]
                    if eng == "A":
                        assert a_name == b_name, (s, "ACT needs a square")
                        nc.scalar.activation(
                            out=dst[:, lo:hi], in_=a[:, lo:hi], func=SQUARE,
                            accum_out=mcol,
                        )
                    elif eng == "D":
                        nc.vector.scalar_tensor_tensor(
                            out=dst[:, lo:hi], in0=a[:, lo:hi], scalar=1.0,
                            in1=b[:, lo:hi], op0=MULT, op1=MULT,
                            accum_out=mcol,
                        )
                    elif eng == "G":
                        nc.gpsimd.scalar_tensor_tensor(
                            out=dst[:, lo:hi], in0=a[:, lo:hi], scalar=1.0,
                            in1=b[:, lo:hi], op0=MULT, op1=MULT,
                            accum_out=mcol,
                        )
                    else:
                        raise ValueError(eng)

                # Transpose moments: [128, K] -> [K, 128] PSUM, copy to SBUF
                mt_ps = pt.tile([mcols, P], F32)
                nc.tensor.transpose(mt_ps[:, :], m_t[:, :], ident[:, :])
                mt_sb = mtsb.tile([mcols, P], F32)
                if cfg["mt"] == "D":
                    nc.vector.tensor_copy(mt_sb[:, :], mt_ps[:, :])
                else:
                    nc.scalar.copy(mt_sb[:, :], mt_ps[:, :])

                # out[128, 1024] = MT.T @ GT  (contraction K)
                o_ps = pout.tile([P, OUT_F], F32)
                for h in range(2):
                    nc.tensor.matmul(
                        o_ps[:, h * 512:(h + 1) * 512],
                        lhsT=mt_sb[:, :],
                        rhs=gt_sb[:, h * 512:(h + 1) * 512],
                        start=True, stop=True,
                    )
                o_sb = ostage.tile([P, OUT_F], F32)
                ca = int(round(OUT_F * cfg["oc_act"] / 128.0)) * 128
                ca = max(0, min(OUT_F, ca))
                if ca > 0:
                    nc.scalar.copy(o_sb[:, 0:ca], o_ps[:, 0:ca])
                if ca < OUT_F:
                    nc.vector.tensor_copy(o_sb[:, ca:OUT_F], o_ps[:, ca:OUT_F])
                nc.sync.dma_start(out=out[it * P:(it + 1) * P, :], in_=o_sb[:, :])

    nc.finalize()
    return nc


_NC_CACHE: dict[tuple, bass.Bass] = {}


def _host_gt(coeffs, cfg=CFG):
    C = _cheb_monomial_matrix()
    G = (coeffs.astype(np.float64) @ C).astype(np.float32)  # [OUT_F, W]
    rows = _moment_rows(cfg)
    GT = np.ascontiguousarray(G.T[rows, :])  # [K, OUT_F]
    return GT


def _run(x, coeffs, input_scale, cfg=CFG, **spmd_kwargs):
    x = np.ascontiguousarray(np.asarray(x, dtype=np.float32))
    coeffs = np.asarray(coeffs, dtype=np.float32)
    scale = float(np.clip(np.asarray(input_scale, dtype=np.float32), 0.1, 2.0).reshape(-1)[0])

    GT = _host_gt(coeffs, cfg)

    key = (scale, str(cfg))
    nc = _NC_CACHE.get(key)
    if nc is None:
        nc = _build_nc(scale, cfg)
        _NC_CACHE[key] = nc

    in_maps = [
        {"x": np.ascontiguousarray(x[c * ROWS_PER_CORE:(c + 1) * ROWS_PER_CORE]),
         "gt": GT}
        for c in range(N_CORES)
    ]
    res = run_bass_kernel_spmd(nc, in_maps, core_ids=list(range(N_CORES)), **spmd_kwargs)
    out = np.concatenate([res.results[c]["out"] for c in range(N_CORES)], axis=0)
    return out.astype(np.float32), res


def kernel(x, coeffs, input_scale):
    out, _ = _run(x, coeffs, input_scale)
    return out


if __name__ == "__main__":
    rng = np.random.default_rng(0)
    x = rng.standard_normal((BATCH, IN_F), dtype=np.float32)
    coeffs = (rng.standard_normal((OUT_F, W)) * 0.1).astype(np.float32)
    s = np.ones((1,), np.float32)
    out = kernel(x=x, coeffs=coeffs, input_scale=s)
    print(out.shape, out.dtype)


# revision 15
# speedup vs baseline: 1.0083x; 1.0083x over previous
"""Trainium2 Bass kernel for ChebyshevActivation.

Math:
    scale = clip(input_scale, 0.1, 2.0)
    t = tanh(x * scale)                        # t in (-1, 1)
    out[b, o] = sum_w coeffs[o, w] * sum_i T_w(t[b, i])

Since |t| < 1, all Chebyshev T_n(t) lie in [-1, 1] and the reference's
clip(+-100) is dead code.  We work in the monomial basis: with power sums
M_j[b] = sum_i t[b,i]^j (M_0 = IN_F exactly) and G = coeffs @ C (C the
Chebyshev->monomial matrix), out = M @ G^T.

Power-sum extraction is one fused pass per "piece": ACT squares with
accum_out, or DVE scalar_tensor_tensor with accum_out (this environment's
walrus rejects TensorScalarPtr on GPSIMD and raw-ISA custom-DVE encodings
from plain Bass, so pieces live on ACT/DVE and the module is built with
Bacc).  Every moment can be split column-wise into pieces on different
engines; each piece accumulates into its own column of the per-tile moment
matrix, and the host duplicates the matching G rows so the final PE matmul
(K = #pieces+1) re-merges them.  Channels t1..t4 are fp16 (bf16 loses too
much precision through the basis change; fp16 keeps DVE 2x modes).

Per-core layout: data-parallel over batch, 8 cores x 1024 rows,
8 row-tiles of [128, 2048] per core.
"""

import numpy as np

import concourse.bass as bass
import concourse.bacc as bacc
import concourse.mybir as mybir
import concourse.tile as tile
from concourse import masks
from concourse.bass_utils import run_bass_kernel_spmd

# This environment's walrus build rejects raw client-encoded ISA instructions
# ("ISA wrong length" for the 64-byte EVENT_SEMAPHORE_RANGE_CLEAR emitted by
# the TileContext exit barrier).  Replace the range-clear with per-semaphore
# EventSemaphore writes (update_mode=sem-wr-imm, value 0), which this walrus
# accepts, so re-executing the loaded NEFF still sees cleared semaphores.
def _sem_clear_via_events(self, sem_range):
    inst = None
    for s in sem_range:
        inst = mybir.InstEventSemaphore(
            name=self.bass.get_next_instruction_name(),
            ins=[], outs=[],
            sync_info=mybir.SyncInfo(
                on_wait=[],
                on_update=[mybir.SyncUpdate(
                    sync_type="semaphore", id=s,
                    update_mode="sem-wr-imm", update_value=0,
                )],
            ),
        )
        self.add_instruction(inst)
    return inst


bass.BassGpSimd.sem_clear = _sem_clear_via_events

N_CORES = 8
BATCH = 8192
IN_F = 2048
OUT_F = 1024
DEG = 8
W = DEG + 1  # 9 moments
ROWS_PER_CORE = BATCH // N_CORES  # 1024
P = 128
NTILES = ROWS_PER_CORE // P  # 8

F32 = mybir.dt.float32
F16 = mybir.dt.float16
MULT = mybir.AluOpType.mult
ADD = mybir.AluOpType.add
SQUARE = mybir.ActivationFunctionType.Square
TANH = mybir.ActivationFunctionType.Tanh

# Stream definitions: name -> (in0, in1, value_dst) ; value_dst None => junk.
# in0 == in1 means the stream is a square (ACT-eligible).
STREAMS = {
    "t2": ("t1", "t1", "t2"),
    "t3": ("t2", "t1", "t3"),
    "t4": ("t2", "t2", "t4"),
    "M5": ("t4", "t1", None),
    "M6": ("t3", "t3", None),
    "M7": ("t4", "t3", None),
    "M8": ("t4", "t4", None),
}
STREAM_MOMENT = {"t2": 2, "t3": 3, "t4": 4, "M5": 5, "M6": 6, "M7": 7, "M8": 8}

# Engine assignment config: stream -> list of (engine, fraction).
# Engines: "A" = ACT square (squares only), "D" = DVE TTR, "G" = GPSIMD stt.
CFG = {
    "t2": [("D", 1.0)],
    "t3": [("D", 1.0)],
    "t4": [("A", 1.0)],
    "M5": [("D", 1.0)],
    "M6": [("A", 1.0)],
    "M7": [("D", 1.0)],
    "M8": [("A", 1.0)],
    "oc_act": 1.0,   # fraction of the PSUM->SBUF output copy done on ACT
    "mt": "D",       # moment-transpose PSUM->SBUF copy engine
    "xin_bufs": 4,
    "chan_bufs": 2,
    "ostage_bufs": 3,
}


def _cheb_monomial_matrix(deg=DEG):
    C = np.zeros((deg + 1, deg + 1), dtype=np.float64)
    C[0, 0] = 1.0
    if deg >= 1:
        C[1, 1] = 1.0
    for n in range(2, deg + 1):
        C[n, 1:] = 2.0 * C[n - 1, :-1]
        C[n, :] -= C[n - 2, :]
    return C


def _pieces(cfg):
    """Deterministic piece list: (stream, engine, col_lo, col_hi)."""
    out = []
    enabled = cfg.get("only_streams")
    for s in STREAMS:
        if enabled is not None and s not in enabled:
            continue
        cols = 0
        parts = cfg[s]
        for idx, (eng, frac) in enumerate(parts):
            if idx == len(parts) - 1:
                hi = IN_F
            else:
                hi = cols + int(round(IN_F * frac / 128.0)) * 128
                hi = min(hi, IN_F)
            if hi > cols:
                out.append((s, eng, cols, hi))
            cols = hi
    return out


def _moment_rows(cfg):
    """Row j of GT corresponds to these moments: [0 (M0), 1 (M1 tanh), *pieces]."""
    rows = [0, 1]
    for s, _eng, _lo, _hi in _pieces(cfg):
        rows.append(STREAM_MOMENT[s])
    return rows


def _build_nc(scale: float, cfg=CFG) -> bass.Bass:
    pieces = _pieces(cfg)
    K = 2 + len(pieces)  # M0 + M1 + pieces
    assert K <= 24
    mcols = K

    nc = bacc.Bacc("TRN2")
    x = nc.dram_tensor("x", [ROWS_PER_CORE, IN_F], F32, kind="ExternalInput")
    gt = nc.dram_tensor("gt", [K, OUT_F], F32, kind="ExternalInput")
    out = nc.dram_tensor("out", [ROWS_PER_CORE, OUT_F], F32, kind="ExternalOutput")

    with tile.TileContext(nc) as tc:
        with (
            tc.tile_pool(name="singles", bufs=1) as singles,
            tc.tile_pool(name="xin", bufs=cfg["xin_bufs"]) as xin,
            tc.tile_pool(name="chan", bufs=cfg["chan_bufs"]) as chan,
            tc.tile_pool(name="junk", bufs=1) as junkp,
            tc.tile_pool(name="mpool", bufs=4) as mpool,
            tc.tile_pool(name="mtsb", bufs=4) as mtsb,
            tc.tile_pool(name="ostage", bufs=cfg["ostage_bufs"]) as ostage,
            tc.tile_pool(name="pt", bufs=2, space="PSUM") as pt,
            tc.tile_pool(name="pout", bufs=2, space="PSUM") as pout,
        ):
            gt_sb = singles.tile([K, OUT_F], F32)
            nc.sync.dma_start(out=gt_sb[:, :], in_=gt[:, :])
            ident = singles.tile([P, P], F32)
            masks.make_identity(nc, ident[:, :])

            j_dve = junkp.tile([P, IN_F], F16, tag="jd")
            j_act = junkp.tile([P, IN_F], F16, tag="ja")
            j_gps = junkp.tile([P, IN_F], F16, tag="jg")
            JUNK = {"A": j_act, "D": j_dve, "G": j_gps}

            for it in range(NTILES):
                x_t = xin.tile([P, IN_F], F32)
                nc.sync.dma_start(out=x_t[:, :], in_=x[it * P:(it + 1) * P, :])

                m_t = mpool.tile([P, mcols], F32)
                nc.gpsimd.memset(m_t[:, 0:1], float(IN_F))

                t1 = chan.tile([P, IN_F], F16, tag="t1")
                t2 = chan.tile([P, IN_F], F16, tag="t2")
                t3 = chan.tile([P, IN_F], F16, tag="t3")
                t4 = chan.tile([P, IN_F], F16, tag="t4")
                VALS = {"t1": t1, "t2": t2, "t3": t3, "t4": t4}

                # t1 = tanh(scale * x), accum -> M1 (col 1)
                nc.scalar.activation(
                    out=t1[:, :], in_=x_t[:, :], func=TANH,
                    scale=scale, accum_out=m_t[:, 1:2],
                )

                for pidx, (s, eng, lo, hi) in enumerate(pieces):
                    a_name, b_name, dst_name = STREAMS[s]
                    a = VALS[a_name]
                    b = VALS[b_name]
                    dst = VALS[dst_name] if dst_name else JUNK[eng]
                    mcol = m_t[:, 2 + pidx:3 + pidx]
                    if eng == "A":
                        assert a_name == b_name, (s, "ACT needs a square")
                        nc.scalar.activation(
                            out=dst[:, lo:hi], in_=a[:, lo:hi], func=SQUARE,
                            accum_out=mcol,
                        )
                    elif eng == "D":
                        nc.vector.scalar_tensor_tensor(
                            out=dst[:, lo:hi], in0=a[:, lo:hi], scalar=1.0,
                            in1=b[:, lo:hi], op0=MULT, op1=MULT,
                            accum_out=mcol,
                        )
                    elif eng == "G":
                        nc.gpsimd.scalar_tensor_tensor(
                            out=dst[:, lo:hi], in0=a[:, lo:hi], scalar=1.0,
                            in1=b[:, lo:hi], op0=MULT, op1=MULT,
                            accum_out=mcol,
                        )
                    else:
                        raise ValueError(eng)

                # Transpose moments: [128, K] -> [K, 128] PSUM, copy to SBUF
                mt_ps = pt.tile([mcols, P], F32)
                nc.tensor.transpose(mt_ps[:, :], m_t[:, :], ident[:, :])
                mt_sb = mtsb.tile([mcols, P], F32)
                if cfg["mt"] == "D":
                    nc.vector.tensor_copy(mt_sb[:, :], mt_ps[:, :])
                else:
                    nc.scalar.copy(mt_sb[:, :], mt_ps[:, :])

                # out[128, 1024] = MT.T @ GT  (contraction K)
                o_ps = pout.tile([P, OUT_F], F32)
                for h in range(2):
                    nc.tensor.matmul(
                        o_ps[:, h * 512:(h + 1) * 512],
                        lhsT=mt_sb[:, :],
                        rhs=gt_sb[:, h * 512:(h + 1) * 512],
                        start=True, stop=True,
                    )
                o_sb = ostage.tile([P, OUT_F], F32)
                ca = int(round(OUT_F * cfg["oc_act"] / 128.0)) * 128
                ca = max(0, min(OUT_F, ca))
                if ca > 0:
                    nc.scalar.copy(o_sb[:, 0:ca], o_ps[:, 0:ca])
                if ca < OUT_F:
                    nc.vector.tensor_copy(o_sb[:, ca:OUT_F], o_ps[:, ca:OUT_F])
                nc.sync.dma_start(out=out[it * P:(it + 1) * P, :], in_=o_sb[:, :])

    nc.finalize()
    return nc


_NC_CACHE: dict[tuple, bass.Bass] = {}


def _host_gt(coeffs, cfg=CFG):
    C = _cheb_monomial_matrix()
    G = (coeffs.astype(np.float64) @ C).astype(np.float32)  # [OUT_F, W]
    rows = _moment_rows(cfg)
    GT = np.ascontiguousarray(G.T[rows, :])  # [K, OUT_F]
    return GT


def _run(x, coeffs, input_scale, cfg=CFG, **spmd_kwargs):
    x = np.ascontiguousarray(np.asarray(x, dtype=np.float32))
    coeffs = np.asarray(coeffs, dtype=np.float32)
    scale = float(np.clip(np.asarray(input_scale, dtype=np.float32), 0.1, 2.0).reshape(-1)[0])

    GT = _host_gt(coeffs, cfg)

    key = (scale, str(cfg))
    nc = _NC_CACHE.get(key)
    if nc is None:
        nc = _build_nc(scale, cfg)
        _NC_CACHE[key] = nc

    in_maps = [
        {"x": np.ascontiguousarray(x[c * ROWS_PER_CORE:(c + 1) * ROWS_PER_CORE]),
         "gt": GT}
        for c in range(N_CORES)
    ]
    res = run_bass_kernel_spmd(nc, in_maps, core_ids=list(range(N_CORES)), **spmd_kwargs)
    out = np.concatenate([res.results[c]["out"] for c in range(N_CORES)], axis=0)
    return out.astype(np.float32), res


def kernel(x, coeffs, input_scale):
    out, _ = _run(x, coeffs, input_scale)
    return out


if __name__ == "__main__":
    rng = np.random.default_rng(0)
    x = rng.standard_normal((BATCH, IN_F), dtype=np.float32)
    coeffs = (rng.standard_normal((OUT_F, W)) * 0.1).astype(np.float32)
    s = np.ones((1,), np.float32)
    out = kernel(x=x, coeffs=coeffs, input_scale=s)
    print(out.shape, out.dtype)
